# revision 3
# baseline (speedup 1.0000x reference)
"""Trainium2 Bass kernel: scatter rho[b, i, j] -> out[b, fock_idx[i], fock_idx[j]].

Sharding: batch dim B across the 8 NeuronCores (pure data parallel). fock_idx is
known on the host at call time, so the scatter addressing is baked into the
compiled program as static DMA/compute access patterns.

Per-core algorithm (out is [D, D], zero except out[idx[i], idx[j]] = rho[i, j]):
  - The runtime hands the NEFF a zero-initialized ExternalOutput buffer, so
    only rows/columns that receive data are written.
  - fock_idx decomposes into 32 runs of 32 consecutive indices spanning
    [c0, c1).  Each rho row is expanded into a [span]-wide row in SBUF with
    the runs at their target offsets and zeros in the gaps; each out row-run
    is stored with one DMA touching columns [c0, c1) only.
  - The DMA engines (16 per core, ~22.5 GB/s each) are the bottleneck:
    ~4.2 MB of loads + ~8.3 MB of span stores = ~34.6 us of engine time.
    To keep them saturated end-to-end: loads pack 2 rho rows per SBUF
    partition (8 KB descriptors, double the per-queue packet throughput),
    are split over the qSP HWDGE ring and the qPool SWDGE ring, and are all
    issued up front; stores alternate between the qSP and qAct HWDGE rings.
  - W expansion buffers are zeroed once (Vector/GpSimd halves) and reused;
    gap columns stay zero across reuse because copies only write data
    columns.  Expansion copies are pair-merged (2 runs per instruction) and
    split across Vector, GpSimd and Scalar(Act activation-copy).
"""

import numpy as np

import concourse.bacc as bacc
import concourse.bass as bass
import concourse.mybir as mybir
from concourse import tile
from concourse.bass_utils import run_bass_kernel_spmd

N_CORES = 8
P = 128  # SBUF partitions


def _runs(dst, src):
    """Maximal runs where dst and src both advance by 1. Yields (d0, s0, len)."""
    out = []
    d0, s0, L = int(dst[0]), int(src[0]), 1
    for k in range(1, len(dst)):
        if int(dst[k]) == d0 + L and int(src[k]) == s0 + L:
            L += 1
        else:
            out.append((d0, s0, L))
            d0, s0, L = int(dst[k]), int(src[k]), 1
    out.append((d0, s0, L))
    return out


def _pair_runs(col_runs):
    """Group adjacent equal-length runs into stride-2 pairs.

    Returns a list of (dst0, src0, pair_dst_stride, pair_src_stride, n, L)
    where n is 1 or 2 repeats of an L-wide copy.
    """
    out = []
    k = 0
    while k < len(col_runs):
        d0, s0, L = col_runs[k]
        if k + 1 < len(col_runs) and col_runs[k + 1][2] == L:
            d1, s1, _ = col_runs[k + 1]
            out.append((d0, s0, d1 - d0, s1 - s0, 2, L))
            k += 2
        else:
            out.append((d0, s0, L, L, 1, L))
            k += 1
    return out


def _plan_tiles(n, row_runs):
    """Tile plan: (r0, rows, rpp, wbuf, pbase). rpp = rho rows per partition."""
    ok2 = n % 2 == 0 and all(s % 2 == 0 and L % 2 == 0 for _, s, L in row_runs)
    if n == 1024 and ok2:
        return [
            (0, 128, 2, "A", 0),
            (128, 128, 2, "A", 64),
            (256, 256, 2, "B", 0),
            (512, 256, 2, "A", 0),
            (768, 256, 2, "B", 0),
        ]
    tiles = []
    r0 = 0
    k = 0
    while r0 < n:
        rows = min(P, n - r0)
        tiles.append((r0, rows, 1, "A" if k % 2 == 0 else "B", 0))
        r0 += rows
        k += 1
    return tiles


def _build(idx, D, n):
    """Build the per-core Bass program with idx baked in."""
    f32 = mybir.dt.float32

    order = np.argsort(idx, kind="stable")
    col_runs = _runs(idx[order], order)  # (dst_col, src_col, len)
    c0 = min(r[0] for r in col_runs)
    c1 = max(r[0] + r[2] for r in col_runs)
    span = c1 - c0
    pairs = _pair_runs(col_runs)

    row_runs = _runs(idx, range(n))
    tiles = _plan_tiles(n, row_runs)
    rpp_max = max(t[2] for t in tiles)

    # Copy split: tiles 0/1 avoid GpSimd (its Q7 is busy issuing SWDGE
    # loads early); later tiles use all three copy-capable engines.
    def copy_engines(nc, t):
        npair = len(pairs)
        if t < 2:
            cut = (npair * 5) // 8
            return [(nc.vector, range(0, cut)), (nc.scalar, range(cut, npair))]
        a = (npair * 3) // 8
        b = (npair * 6) // 8
        return [(nc.vector, range(0, a)), (nc.gpsimd, range(a, b)),
                (nc.scalar, range(b, npair))]

    nc = bacc.Bacc("TRN2", target_bir_lowering=False, debug=False,
                   num_devices=N_CORES)
    rho = nc.dram_tensor("rho", [n, n], f32, kind="ExternalInput")
    out = nc.dram_tensor("out", [D, D], f32, kind="ExternalOutput")
    rho_flat = rho[:, :]

    with tile.TileContext(nc) as tc:
        with (
            tc.tile_pool(name="rp", bufs=1) as rp,
            tc.tile_pool(name="wp", bufs=1) as wp,
        ):
            ws = {
                "A": wp.tile([P, rpp_max * span], f32, name="WA"),
                "B": wp.tile([P, rpp_max * span], f32, name="WB"),
            }
            rs = [rp.tile([P, t[2] * n], f32, name=f"R{k}")
                  for k, t in enumerate(tiles)]

            def load_ap(t):
                r0, rows, rpp, _, _ = tiles[t]
                parts = rows // rpp
                src = bass.AP(rho_flat.tensor, rho_flat.offset + r0 * n,
                              [[rpp * n, parts], [1, rpp * n]])
                return rs[t][:parts, :], src

            # Issue loads up front: L0 on the SP HWDGE ring (its stores only
            # queue behind this one small load), the rest on the Pool SWDGE
            # ring.  W zero-fills interleave so WA is ready for tile 0 and
            # WB for tile 2 without delaying load issue.
            d, s = load_ap(0)
            nc.sync.dma_start(d, s)
            d, s = load_ap(1)
            nc.gpsimd.dma_start(d, s)
            nc.vector.memset(ws["A"][:, :span], 0.0)
            if rpp_max > 1:
                nc.gpsimd.memset(ws["A"][:, span:], 0.0)
            for t in range(2, len(tiles)):
                d, s = load_ap(t)
                nc.gpsimd.dma_start(d, s)
                if t == 3 and rpp_max > 1:
                    nc.gpsimd.memset(ws["B"][:, span:], 0.0)
            if len(tiles) <= 3 and rpp_max > 1:
                nc.gpsimd.memset(ws["B"][:, span:], 0.0)

            n_store = 0
            for t, (r0, rows, rpp, wname, pbase) in enumerate(tiles):
                parts = rows // rpp
                W = ws[wname]
                R = rs[t]
                if t == 1:
                    # WB's left half zeroed after tile 0's Vector copies so
                    # it does not delay the first stores; ready before
                    # tile 2 (the first WB user) needs it.
                    nc.vector.memset(ws["B"][:, :span], 0.0)

                # Expansion copies: runs placed at target offsets, 2 runs
                # per instruction where possible, all rpp sub-rows at once.
                for eng, rng in copy_engines(nc, t):
                    for k in rng:
                        d0, s0, ds, ss, cnt, L = pairs[k]
                        doff = W.offset + pbase * W.ap[0][0] + (d0 - c0)
                        soff = R.offset + s0
                        if cnt == 1:
                            dst = bass.AP(W.tensor, doff,
                                          [[W.ap[0][0], parts],
                                           [span, rpp], [1, L]])
                            src = bass.AP(R.tensor, soff,
                                          [[R.ap[0][0], parts],
                                           [n, rpp], [1, L]])
                        else:
                            dst = bass.AP(W.tensor, doff,
                                          [[W.ap[0][0], parts], [span, rpp],
                                           [ds, cnt], [1, L]])
                            src = bass.AP(R.tensor, soff,
                                          [[R.ap[0][0], parts], [n, rpp],
                                           [ss, cnt], [1, L]])
                        if eng is nc.scalar:
                            eng.copy(dst, src)
                        else:
                            eng.tensor_copy(dst, src)

                # Row-run stores for this tile, alternating HWDGE rings.
                for dr, sr, Lr in _runs(idx[r0:r0 + rows], range(rows)):
                    ring = nc.sync if n_store % 2 == 0 else nc.scalar
                    n_store += 1
                    if rpp == 1:
                        ring.dma_start(out[dr:dr + Lr, c0:c1],
                                       W[pbase + sr:pbase + sr + Lr, :span])
                        continue
                    if sr % rpp == 0 and Lr % rpp == 0:
                        p0 = pbase + sr // rpp
                        src = bass.AP(W.tensor, W.offset + p0 * W.ap[0][0],
                                      [[W.ap[0][0], Lr // rpp],
                                       [span, rpp], [1, span]])
                        ring.dma_start(out[dr:dr + Lr, c0:c1], src)
                    else:
                        for j in range(Lr):
                            p0 = pbase + (sr + j) // rpp
                            sub = (sr + j) % rpp
                            src = bass.AP(W.tensor,
                                          W.offset + p0 * W.ap[0][0]
                                          + sub * span,
                                          [[W.ap[0][0], 1], [1, span]])
                            r2 = nc.sync if n_store % 2 == 0 else nc.scalar
                            n_store += 1
                            r2.dma_start(out[dr + j:dr + j + 1, c0:c1], src)
    nc.compile()
    return nc


def kernel(input_state, fock_idx, fock_dim):
    input_state = np.asarray(input_state)
    idx = np.asarray(fock_idx).astype(np.int64)
    D = int(fock_dim)
    B, n, _ = input_state.shape

    nc = _build(idx, D, n)

    out = np.empty((B, D, D), dtype=input_state.dtype)
    for start in range(0, B, N_CORES):
        stop = min(start + N_CORES, B)
        in_maps = [
            {"rho": np.ascontiguousarray(input_state[b], dtype=np.float32)}
            for b in range(start, stop)
        ]
        res = run_bass_kernel_spmd(nc, in_maps,
                                   core_ids=list(range(stop - start)))
        for k, b in enumerate(range(start, stop)):
            out[b] = res.results[k]["out"]
    return out
